# revision 2
# baseline (speedup 1.0000x reference)
"""Trainium2 Bass kernel for nn_AvgPoolCustom.

Reference computation:
    out[b,c,j,i] = sum_{a,r} features[b,c,a,r] * A[a,j] * R[b,i,r]
with A the arc-coverage matrix (integer boundaries 90/180/270/360 -> exact
mean over 4 contiguous groups of 90 arcs, i.e. A[a,j] = 1/90 for a in group j)
and R the per-batch ring-coverage matrix (tiny: 5 x 180, derived from bsize_x).

Strategy (pure data parallel over batch, 2 batches per core on 8 cores):
  host:   compute W[b,i,r] = R[b,i,r] / 90  (float64 -> float32), replicate
          across 128 partitions.
  device: per (batch, channel-block of 128, arc-group of 90):
            DMA the (128, 90*180) chunk to SBUF (contiguous per partition),
            s[c,r] = sum_a chunk[c,a,r]   (one strided vector tensor_reduce)
          then per (group j, ring i):
            out[c, j*5+i] = sum_r s_j[c,r] * W[b,i,r]   (tensor_tensor_reduce)
"""

import numpy as np

B, C, NA, NR = 16, 256, 360, 180
N_CORES = 8
BPC = B // N_CORES          # batches per core
KA, KR = 4, 5               # arc groups / ring rows
GA = NA // KA               # arcs per group = 90
RINGS = np.array([1.0, 2.0, 3.0, 4.0, 6.0], dtype=np.float64)

_CACHE = {}


def _ring_weights(bsize_x: np.ndarray) -> np.ndarray:
    """W[b,i,r] = R[b,i,r] / GA, float32, shape (B, KR, NR)."""
    bs = bsize_x.astype(np.float64)
    mm_per_pixel = bs * 0.1 / 2.0 / NR                       # (B,)
    rpx = np.clip(RINGS[None, :] / mm_per_pixel[:, None], 0.0, float(NR))  # (B,KR)
    lower = np.concatenate([np.zeros((rpx.shape[0], 1)), rpx[:, :-1]], axis=1)
    idx = np.arange(NR, dtype=np.float64)
    cov = np.clip(
        np.minimum(rpx[..., None], idx + 1.0) - np.maximum(lower[..., None], idx),
        0.0, None,
    )                                                        # (B,KR,NR)
    s = cov.sum(axis=-1, keepdims=True)
    R = cov / np.where(s == 0, 1.0, s)
    return (R / GA).astype(np.float32)


def _build():
    import concourse.bacc as bacc
    import concourse.mybir as mybir
    from concourse.tile import TileContext

    f32 = mybir.dt.float32
    nc = bacc.Bacc(
        "TRN2",
        target_bir_lowering=False,
        debug=False,
        enable_asserts=False,
        num_devices=N_CORES,
    )
    feat = nc.dram_tensor("features", (BPC, C, NA, NR), f32, kind="ExternalInput").ap()
    rmat = nc.dram_tensor("rmat", (BPC, 128, KR * NR), f32, kind="ExternalInput").ap()
    out = nc.dram_tensor("out", (BPC, C, KA * KR), f32, kind="ExternalOutput").ap()

    with TileContext(nc) as tc:
        with (
            tc.tile_pool(name="chunk", bufs=2) as chunk_pool,
            tc.tile_pool(name="rb", bufs=2) as rb_pool,
            tc.tile_pool(name="stile", bufs=2) as s_pool,
            tc.tile_pool(name="scratch", bufs=4) as scratch_pool,
            tc.tile_pool(name="otile", bufs=2) as o_pool,
        ):
            for b in range(BPC):
                rb = rb_pool.tile([128, KR * NR], f32)
                nc.sync.dma_start(out=rb[:], in_=rmat[b])
                for cb in range(C // 128):
                    cs = cb * 128
                    s_tile = s_pool.tile([128, KA * NR], f32)
                    for j in range(KA):
                        x = chunk_pool.tile([128, GA * NR], f32)
                        src = feat[b, cs:cs + 128, j * GA:(j + 1) * GA, :]
                        nc.sync.dma_start(out=x[:], in_=src.rearrange("c a r -> c (a r)"))
                        xv = x[:].rearrange("c (a r) -> c r a", a=GA)
                        nc.vector.reduce_sum(
                            out=s_tile[:, j * NR:(j + 1) * NR],
                            in_=xv,
                            axis=mybir.AxisListType.X,
                        )
                    ot = o_pool.tile([128, KA * KR], f32)
                    for j in range(KA):
                        for i in range(KR):
                            sc = scratch_pool.tile([128, NR], f32)
                            nc.vector.scalar_tensor_tensor(
                                out=sc[:],
                                in0=s_tile[:, j * NR:(j + 1) * NR],
                                scalar=1.0,
                                in1=rb[:, i * NR:(i + 1) * NR],
                                op0=mybir.AluOpType.mult,
                                op1=mybir.AluOpType.mult,
                                accum_out=ot[:, j * KR + i:j * KR + i + 1],
                            )
                    nc.sync.dma_start(out=out[b, cs:cs + 128], in_=ot[:])

    nc.compile()
    return nc


def _get_nc():
    if "nc" not in _CACHE:
        _CACHE["nc"] = _build()
    return _CACHE["nc"]


def run(features: np.ndarray, bsize_x: np.ndarray, trace: bool = False):
    from concourse import bass_utils

    nc = _get_nc()
    W = _ring_weights(bsize_x)                               # (B, KR, NR)
    in_maps = []
    for k in range(N_CORES):
        fshard = np.ascontiguousarray(features[k * BPC:(k + 1) * BPC])
        wshard = np.ascontiguousarray(
            np.broadcast_to(
                W[k * BPC:(k + 1) * BPC].reshape(BPC, 1, KR * NR), (BPC, 128, KR * NR)
            )
        )
        in_maps.append({"features": fshard, "rmat": wshard})
    res = bass_utils.run_bass_kernel_spmd(
        nc, in_maps, core_ids=list(range(N_CORES)), trace=trace
    )
    parts = [res.results[k]["out"].reshape(BPC, C, KA, KR) for k in range(N_CORES)]
    return np.concatenate(parts, axis=0), res


def kernel(features: np.ndarray, bsize_x: np.ndarray) -> np.ndarray:
    out, _ = run(np.asarray(features), np.asarray(bsize_x), trace=False)
    return out
